# revision 1
# baseline (speedup 1.0000x reference)
"""Trainium2 Bass kernel v2 for nn_AttnBlock (sparse GQA attention block).

Sharding: 8 cores = batch(2) x head-group(4). Device program:
- qkv projection in dj-outer sweeps of 4 token tiles (PE consumes x slices as
  the DMA delivers them); token rms stats matmuls interleaved into sweep 1.
- rope on raw q/k; q rms scale applied post-rope (Pool); k rms scale and the
  1/sqrt(dh) score scale folded into the exp eviction's per-partition scale AP
  (exp partitions = k tokens).
- attention per (qt,kt) strip [h0|h2|h1|h3]: 2-head score matmuls + one 2-head
  stride-0 mask add per partition half; exp per strip; all-ones denominator;
  reciprocal + normalize on DVE; PV batched per half.
- attention for early q tiles is emitted interleaved with the rope of late
  tiles so DVE/ACT attention work overlaps phase A's tail.
- output projection per 512-token chunk, bf16 partials (host sums in f32).
"""

import sys
from contextlib import ExitStack

try:
    import concourse.bass  # noqa: F401  (provided by the axon site tree)
except ImportError:
    sys.path.insert(0, "/opt/trn_rl_repo")

import numpy as np
import ml_dtypes

import concourse.bass as bass
import concourse.tile as tile
import concourse.mybir as mybir
from concourse.masks import make_identity

F32 = mybir.dt.float32
BF16 = mybir.dt.bfloat16
BF = ml_dtypes.bfloat16

B, L, D = 2, 2048, 1024
HEADS, KV_HEADS, DH = 16, 4, 64
WINDOW = 1024
NEG = -1e30
EPS = 1.1920929e-07
NT = L // 128          # 16 token tiles
ND = D // 128          # 8 d tiles
NG = 4                 # head groups (= cores per batch)
SCALE = 1.0 / np.sqrt(DH)
SX = 16.0              # fp8 pre-scale for x
SW = 64.0              # fp8 pre-scale for w_qkv
SXW = SX * SW
LN_SCALE = float(np.log(SCALE) - np.log(SXW))   # k-head exp bias (incl 1/S)
MLN_S = float(-np.log(SXW))                     # q-head exp bias (1/S)
ACT = mybir.ActivationFunctionType


def split_multi_waits(nc):
    """This environment's walrus supports only ONE sync wait per instruction.
    Split each multi-wait instruction into single-wait NoOps inserted just
    before it (same engine; per-engine execution is in-order, so consecutive
    single waits are equivalent to one multi-wait)."""
    for func in nc.m.functions:
        for block in func.blocks:
            new_list = []
            for inst in block.instructions:
                si = inst.sync_info
                if si is not None and len(si.on_wait) > 1:
                    waits = list(si.on_wait)
                    for w in waits[:-1]:
                        new_list.append(mybir.InstNoOp(
                            name=f"waitsplit-{nc.next_id()}",
                            engine=inst.engine,
                            sync_info=mybir.SyncInfo(on_wait=[w], on_update=[]),
                            text_hint="waitsplit", bass_nofuse=True))
                    inst.sync_info = mybir.SyncInfo(
                        on_wait=[waits[-1]], on_update=list(si.on_update))
                if si is not None and len(si.on_update) > 1:
                    ups = list(inst.sync_info.on_update)
                    inst.sync_info = mybir.SyncInfo(
                        on_wait=list(inst.sync_info.on_wait), on_update=[ups[0]])
                    new_list.append(inst)
                    for u in ups[1:]:
                        new_list.append(mybir.InstNoOp(
                            name=f"updsplit-{nc.next_id()}",
                            engine=inst.engine,
                            sync_info=mybir.SyncInfo(on_wait=[], on_update=[u]),
                            text_hint="updsplit", bass_nofuse=True))
                    continue
                new_list.append(inst)
            block.instructions[:] = new_list


# ---------------------------------------------------------------- host plan

def plan_structure(reset_mask: np.ndarray):
    """Derive the union block-sparse structure and per-batch additive masks."""
    lo = np.zeros((B, L), np.int64)
    idx = np.arange(L)
    for b in range(B):
        r = np.where(np.asarray(reset_mask[b], bool), idx, 0)
        last_reset = np.maximum.accumulate(r)
        lo[b] = np.maximum(last_reset, idx - (WINDOW - 1))

    kts = []
    widths = []
    for qt in range(NT):
        kt_min = min(int(lo[b, 128 * qt] // 128) for b in range(B))
        kts.append(list(range(kt_min, qt + 1)))
        ws = []
        qs = np.arange(128 * qt, 128 * qt + 128)
        for kt in kts[qt]:
            if kt == qt:
                ws.append(128)
            else:
                cross = max(int((lo[b, qs] < 128 * (kt + 1)).sum()) for b in range(B))
                ws.append(min(128, max(32, -(-cross // 32) * 32)))
        widths.append(ws)

    pairs = [(qt, kt) for qt in range(NT) for kt in kts[qt]]
    masks = np.zeros((B, len(pairs), 128, 128), np.float32)
    kk = idx[:128]
    for b in range(B):
        for i, (qt, kt) in enumerate(pairs):
            k = 128 * kt + kk[:, None]             # [128,1] global k
            q = 128 * qt + kk[None, :]             # [1,128] global q
            valid = (k >= lo[b, 128 * qt:128 * qt + 128][None, :]) & (k <= q)
            masks[b, i] = np.where(valid, 0.0, NEG)
    return kts, widths, masks


# ------------------------------------------------------------ device build

def build_program(kts, widths):
    pairs = [(qt, kt) for qt in range(NT) for kt in kts[qt]]
    pair_idx = {p: i for i, p in enumerate(pairs)}
    NP = len(pairs)
    # per-qt strips: diagonal kt first at pT offset 0, then older kts.
    # strip layout along pT: [h0 w | h2 w | h1 w | h3 w]  (pair-half major)
    strips = []            # strips[qt] = list of (kt, w, pT_offset)
    PTmax = 0
    for qt in range(NT):
        ss = []
        off = 0
        for ki in range(len(kts[qt]) - 1, -1, -1):   # diagonal first
            kt, w = kts[qt][ki], widths[qt][ki]
            ss.append((kt, w, off))
            off += 4 * w
        strips.append(ss)
        PTmax = max(PTmax, off)

    nc = bass.Bass("TRN2", target_bir_lowering=False, debug=False, num_devices=8)
    FP8 = mybir.dt.float8e4
    ap_x8 = nc.dram_tensor("x8", [ND, 128, L], FP8, kind="ExternalInput").ap()
    ap_rx8 = nc.dram_tensor("rx8", [ND, 128, L], FP8, kind="ExternalInput").ap()
    ap_w8 = nc.dram_tensor("w8", [ND, 128, 384], FP8, kind="ExternalInput").ap()
    ap_rw8 = nc.dram_tensor("rw8", [ND, 128, 384], FP8, kind="ExternalInput").ap()
    ap_woutP = nc.dram_tensor("woutP", [2, 128, D], BF16, kind="ExternalInput").ap()
    ap_cosF = nc.dram_tensor("cosF", [NT, 128, DH], BF16, kind="ExternalInput").ap()
    ap_sinF2 = nc.dram_tensor("sinF2", [NT, 128, DH], BF16, kind="ExternalInput").ap()
    ap_masks = nc.dram_tensor("masks", [NP, 128, 128], BF16, kind="ExternalInput").ap()
    ap_outT = nc.dram_tensor("outT", [D, L], BF16, kind="ExternalOutput").ap()

    def sb_ap(t, offset_elems, dims):
        return bass.AP(tensor=t.tensor, offset=t.offset + offset_elems,
                       ap=[t.ap[0]] + dims)

    with tile.TileContext(nc) as tc, ExitStack() as ctx:
        csts = ctx.enter_context(tc.tile_pool(name="consts", bufs=1))
        big = ctx.enter_context(tc.tile_pool(name="big", bufs=1))
        dramp = ctx.enter_context(tc.tile_pool(name="dram", bufs=1, space="DRAM"))

        ident = csts.tile([128, 128], BF16, tag="ident")
        allones = csts.tile([128, 128], BF16, tag="allones")
        eps_c = csts.tile([128, 1], F32, tag="eps_c")
        lnsc_c = csts.tile([128, 1], F32, tag="lnsc_c")
        mlns_c = csts.tile([128, 1], F32, tag="mlns_c")
        make_identity(nc, ident)
        nc.gpsimd.memset(allones, 1.0)
        nc.gpsimd.memset(eps_c, EPS)
        nc.gpsimd.memset(lnsc_c, LN_SCALE)
        nc.gpsimd.memset(mlns_c, MLN_S)

        # input loads: x8[0] first, weights, remaining x8 slices, then rx8
        FP8 = mybir.dt.float8e4
        w8_sb = csts.tile([128, ND, 384], FP8, tag="w8")
        rw8_sb = csts.tile([128, ND, 384], FP8, tag="rw8")
        x8_sb = big.tile([128, ND, L], FP8, tag="x8")
        rx8_sb = big.tile([128, ND, L], FP8, tag="rx8")
        nc.sync.dma_start(out=x8_sb[:, 0, :], in_=ap_x8[0])
        nc.sync.dma_start(out=w8_sb, in_=ap_w8.rearrange("n p f -> p n f"))
        nc.sync.dma_start(out=rw8_sb, in_=ap_rw8.rearrange("n p f -> p n f"))
        for dj in range(1, ND):
            nc.sync.dma_start(out=x8_sb[:, dj, :], in_=ap_x8[dj])
            nc.sync.dma_start(out=rx8_sb[:, dj - 1, :], in_=ap_rx8[dj - 1])
        nc.sync.dma_start(out=rx8_sb[:, ND - 1, :], in_=ap_rx8[ND - 1])
        cos_sb = csts.tile([128, NT, DH], BF16, tag="cos")
        nc.sync.dma_start(out=cos_sb, in_=ap_cosF.rearrange("n p f -> p n f"))
        sin_sb = csts.tile([128, NT, DH], BF16, tag="sin")
        nc.sync.dma_start(out=sin_sb, in_=ap_sinF2.rearrange("n p f -> p n f"))
        mask_sb = csts.tile([128, NP, 128], BF16, tag="mask")
        nc.sync.dma_start(out=mask_sb, in_=ap_masks.rearrange("n p f -> p n f"))
        wout_sb = csts.tile([128, 2, D], BF16, tag="wout")
        nc.sync.dma_start(out=wout_sb, in_=ap_woutP.rearrange("n p f -> p n f"))

        qkv_raw = big.tile([128, NT, 6, DH], BF16, tag="qkv_raw")  # 0:5 q/k, 5 v
        qTp = big.tile([128, NT, 2, 128], BF16, tag="qTp")
        kvT = big.tile([128, NT, 128], BF16, tag="kvT")   # kT in BOTH halves
        yTn2 = big.tile([128, 2, NT, 128], BF16, tag="yTn2")
        ms_qk = big.tile([128, NT, 5], F32, tag="ms_qk")
        s_qk = big.tile([128, NT, 5], F32, tag="s_qk")
        ln_qk = big.tile([128, NT, 5], F32, tag="ln_qk")
        ln_cols = big.tile([128, NT], F32, tag="ln_cols")
        s_cols = big.tile([128, NT], F32, tag="s_cols")
        ms_sb = big.tile([1, 4, 512], F32, tag="ms_sb")
        scratch_dram = dramp.tile([L], F32)
        outT_cols = ap_outT.rearrange("(n p) l -> p n l", p=128)

        with tc.tile_pool(name="x2p", bufs=8) as x2p, \
             tc.tile_pool(name="sqp", bufs=2) as sqp, \
             tc.tile_pool(name="qsc", bufs=3) as qsc, \
             tc.tile_pool(name="ptp", bufs=4) as ptp, \
             tc.tile_pool(name="rp", bufs=4) as rp, \
             tc.tile_pool(name="osb", bufs=2) as osb:

            P = {}           # late-bound psum pools for the helper closures
            qkv_ctx = ExitStack()
            qkvps = None

            DR = mybir.MatmulPerfMode.DoubleRow

            def sweep(tis, chunks=(), ms_ps=None, x2_list=None):
                tiles = [P["qkvps"].tile([128, 384], F32, tag=f"qkv{i}",
                                    name=f"qkvps{i}")
                         for i in range(len(tis))]
                terms = [(x8_sb, w8_sb), (x8_sb, rw8_sb), (rx8_sb, w8_sb)]
                NDP = ND // 2
                for dj in range(ND):
                    if x2_list is not None and len(x2_list) <= dj:
                        x2 = x2p.tile([128, L], BF16, tag="x2")
                        x2_list.append(x2)
                        if dj % 2 == 0:
                            nc.scalar.activation(out=x2, in_=x8_sb[:, dj, :],
                                                 func=ACT.Square)
                        else:
                            nc.vector.tensor_mul(x2, x8_sb[:, dj, :],
                                                 x8_sb[:, dj, :])
                    if dj % 2 == 1:
                        djp = dj - 1
                        for t, (xs, ws) in enumerate(terms):
                            for i, ti in enumerate(tis):
                                nc.tensor.matmul(
                                    tiles[i],
                                    xs[:, djp:djp + 2, 128 * ti:128 * ti + 128],
                                    ws[:, djp:djp + 2, :],
                                    start=(djp == 0 and t == 0),
                                    stop=(djp == ND - 2 and t == 2),
                                    perf_mode=DR)
                    for ci, c in enumerate(chunks):
                        nc.tensor.matmul(
                            ms_ps[:, ci, :], allones[:, 0:1],
                            x2_list[dj][:, 512 * c:512 * c + 512],
                            start=(dj == 0), stop=(dj == ND - 1))
                for i, ti in enumerate(tis):
                    nc.scalar.activation(
                        out=qkv_raw[:, ti, :, :],
                        in_=tiles[i].rearrange("p (h d) -> p h d", d=DH),
                        func=ACT.Copy)
                    sq = sqp.tile([128, 5, DH], BF16, tag="sq")
                    nc.gpsimd.tensor_mul(sq, qkv_raw[:, ti, 0:5, :],
                                         qkv_raw[:, ti, 0:5, :])
                    nc.vector.tensor_reduce(out=ms_qk[:, ti, :], in_=sq,
                                            axis=mybir.AxisListType.X,
                                            op=mybir.AluOpType.add)

            def rope_block(tis):
                g = tis[0]
                trps = P["trps"]
                nc.scalar.activation(
                    out=ln_qk[:, g:tis[-1] + 1, :], in_=ms_qk[:, g:tis[-1] + 1, :],
                    func=ACT.Ln, scale=1.0 / (DH * SXW * SXW), bias=eps_c)
                lq = ln_qk.rearrange("p n f -> p (n f)")
                sq_ = s_qk.rearrange("p n f -> p (n f)")
                for t2 in tis:
                    nc.scalar.activation(
                        out=sq_[:, 5 * t2:5 * t2 + 4], in_=lq[:, 5 * t2:5 * t2 + 4],
                        func=ACT.Exp, scale=-0.5, bias=mlns_c)
                    nc.scalar.activation(
                        out=sq_[:, 5 * t2 + 4:5 * t2 + 5],
                        in_=lq[:, 5 * t2 + 4:5 * t2 + 5],
                        func=ACT.Exp, scale=-0.5, bias=lnsc_c)
                for ti in tis:
                    raw = qkv_raw[:, ti, 0:5, :]
                    cos_b = sb_ap(cos_sb, ti * DH, [[0, 5], [1, DH]])
                    sin_b = sb_ap(sin_sb, ti * DH, [[0, 5], [1, DH]])
                    half = DH // 2
                    rswap = sb_ap(qkv_raw, (ti * 6) * DH + half,
                                  [[DH, 5], [-half, 2], [1, half]])
                    ra = qsc.tile([128, 5, DH], BF16, tag="ra")
                    rb = qsc.tile([128, 5, DH], BF16, tag="rb")
                    rot = qsc.tile([128, 5, DH], BF16, tag="rot")
                    nc.vector.tensor_mul(ra, raw, cos_b)
                    nc.vector.tensor_mul(rb, rswap, sin_b)
                    nc.vector.tensor_add(rot, ra, rb)
                    qks = qsc.tile([128, 4, DH], BF16, tag="qks")
                    for hh in range(4):
                        nc.gpsimd.tensor_scalar_mul(
                            qks[:, hh, :], rot[:, hh, :],
                            s_qk[:, ti, hh:hh + 1])
                    tr = trps.tile([128, 4, 128], BF16, tag="tr")
                    nc.tensor.transpose(tr[:, 0, :], qks[:, 0:2, :], ident)
                    nc.tensor.transpose(tr[:, 1, :], qks[:, 2:4, :], ident)
                    nc.tensor.transpose(tr[:, 2, :], rot[:, 3:5, :], ident)
                    trk = tr[0:64, 3, :]
                    nc.tensor.transpose(trk, rot[:, 4, :], ident)
                    nc.scalar.activation(out=qTp[:, ti, :, :], in_=tr[:, 0:2, :],
                                         func=ACT.Copy)
                    nc.vector.tensor_copy(kvT[64:128, ti, :], tr[64:128, 2, :])
                    nc.vector.tensor_copy(kvT[0:64, ti, :], trk)

            def vscale(tis):
                for ti in tis:
                    nc.gpsimd.tensor_scalar_mul(qkv_raw[:, ti, 5, :],
                                                qkv_raw[:, ti, 5, :],
                                                s_cols[:, ti:ti + 1])

            def attn(qt):
                sps, denps, y2ps, ops = P["sps"], P["denps"], P["y2ps"], P["ops"]
                ss = strips[qt]
                pT = ptp.tile([128, PTmax], BF16, tag="pT")
                for kt, w, off in ss:
                    s_ps = sps.tile([128, 512], F32, tag="s_ps")
                    for h in range(2):
                        base = 64 * h
                        reg = s_ps[:, 2 * w * h:2 * w * h + 2 * w]
                        nc.tensor.matmul(
                            reg,
                            kvT[base:base + 64, kt, :],
                            qTp[base:base + 64, qt, :, 0:w],
                            start=True, stop=False)
                        m_bc = sb_ap(mask_sb, pair_idx[(qt, kt)] * 128,
                                     [[0, 2], [1, w]])
                        nc.tensor.matmul(reg, ident, m_bc,
                                         start=False, stop=True)
                    nc.scalar.activation(out=pT[:, off:off + 4 * w],
                                         in_=s_ps[:, 0:4 * w],
                                         func=ACT.Exp,
                                         scale=s_qk[:, kt, 4:5])
                den_ps = denps.tile([128, 512], F32, tag="den")
                for si, (kt, w, off) in enumerate(ss):
                    last = si == len(ss) - 1
                    if w == 128:
                        nc.tensor.matmul(den_ps, allones, pT[:, off:off + 512],
                                         start=(si == 0), stop=last)
                    else:
                        for s in range(4):
                            nc.tensor.matmul(
                                den_ps[:, 128 * s:128 * s + w], allones,
                                pT[:, off + s * w:off + (s + 1) * w],
                                start=False, stop=(last and s == 3))
                y2_ps = y2ps.tile([128, 2, 128], F32, tag="y2")
                y2_f = y2_ps.rearrange("p a b -> p (a b)")
                for h in range(2):
                    base = 64 * h
                    for si, (kt, w, off) in enumerate(ss):
                        last = si == len(ss) - 1
                        if w == 128:
                            nc.tensor.matmul(
                                y2_f[base:base + 64, :],
                                qkv_raw[:, kt, 5, :],
                                pT[:, off + 256 * h:off + 256 * h + 256],
                                start=(si == 0), stop=last)
                        else:
                            for s in range(2):
                                nc.tensor.matmul(
                                    y2_ps[base:base + 64, s, 0:w],
                                    qkv_raw[:, kt, 5, :],
                                    pT[:, off + (2 * h + s) * w:
                                        off + (2 * h + s + 1) * w],
                                    start=False, stop=(last and s == 1))
                r_t = rp.tile([128, 512], F32, tag="r_t")
                nc.vector.reciprocal(out=r_t, in_=den_ps)
                for h in range(2):
                    base = 64 * h
                    r_half = bass.AP(tensor=r_t.tensor,
                                     offset=r_t.offset + 256 * h,
                                     ap=[[r_t.ap[0][0], 64], [128, 2], [1, 128]])
                    dst = yTn2[base:base + 64, :, qt, :]
                    src = y2_ps[base:base + 64, :, :]
                    nc.vector.tensor_mul(dst, src, r_half)

                if qt % 2 == 1:
                    qlo = qt - 1
                    o_col = osb.tile([128, ND, 256], BF16, tag="o_col")
                    for dt in range(ND):
                        o_ps = ops.tile([128, 256], F32, tag="o_ps")
                        for pair in range(2):
                            nc.tensor.matmul(
                                o_ps,
                                wout_sb[:, pair, 128 * dt:128 * dt + 128],
                                yTn2[:, pair, qlo:qlo + 2, :].rearrange(
                                    "p a b -> p (a b)"),
                                start=(pair == 0), stop=(pair == 1))
                        if dt % 4 < 3:
                            nc.scalar.activation(out=o_col[:, dt, :],
                                                 in_=o_ps, func=ACT.Copy)
                        else:
                            nc.vector.tensor_copy(o_col[:, dt, :], o_ps)
                    nc.sync.dma_start(
                        out=outT_cols[:, :, 128 * qlo:128 * qlo + 256],
                        in_=o_col)

            # ---- phase A ------------------------------------------------
            with tc.tile_pool(name="trps", bufs=1, space="PSUM") as trps:
                P["trps"] = trps
                P["qkvps"] = qkv_ctx.enter_context(
                    tc.tile_pool(name="qkvps", bufs=1, space="PSUM"))
                x2s = []
                with tc.tile_pool(name="msps", bufs=1, space="PSUM") as msps:
                    ms_a = msps.tile([1, 2, 512], F32, tag="ms")
                    sweep([0, 1, 2, 3], chunks=(0, 1), ms_ps=ms_a,
                          x2_list=x2s)
                    nc.scalar.activation(out=ms_sb[:, 0:2, :], in_=ms_a,
                                         func=ACT.Copy)
                    ms_b = msps.tile([1, 2, 512], F32, tag="ms")
                    sweep([4, 5, 6, 7], chunks=(2, 3), ms_ps=ms_b,
                          x2_list=x2s)
                    nc.scalar.activation(out=ms_sb[:, 2:4, :], in_=ms_b,
                                         func=ACT.Copy)

                nc.sync.dma_start(out=scratch_dram,
                                  in_=ms_sb.rearrange("p a b -> p (a b)"))
                nc.sync.dma_start(out=s_cols,
                                  in_=scratch_dram.rearrange("(c p) -> p c",
                                                             p=128))
                # rsqrt(v) = exp(-0.5 ln(v)); ACT Rsqrt is blocked by bass
                nc.scalar.activation(out=ln_cols, in_=s_cols, func=ACT.Ln,
                                     scale=1.0 / (D * SX * SX), bias=eps_c)
                nc.scalar.activation(out=s_cols, in_=ln_cols, func=ACT.Exp,
                                     scale=-0.5, bias=mlns_c)
                rope_block([0, 1, 2, 3])
                sweep([8, 9, 10, 11])
                rope_block([4, 5, 6, 7])
                sweep([12, 13, 14, 15])
                qkv_ctx.close()      # free the 4 qkv psum banks

                with tc.tile_pool(name="sps", bufs=3, space="PSUM") as sps, \
                     tc.tile_pool(name="denps", bufs=1, space="PSUM") as denps, \
                     tc.tile_pool(name="y2ps", bufs=1, space="PSUM") as y2ps, \
                     tc.tile_pool(name="ops", bufs=2, space="PSUM") as ops:
                    P.update(sps=sps, denps=denps, y2ps=y2ps, ops=ops)
                    vscale(range(0, 8))
                    rope_block([8, 9, 10, 11])
                    attn(0)
                    attn(1)
                    rope_block([12, 13, 14, 15])
                    attn(2)
                    attn(3)
                    vscale(range(8, NT))
                    for qt in range(4, NT):
                        attn(qt)

    return nc


# ------------------------------------------------------------- host driver

_COS_SIN = None


def _cos_sin():
    global _COS_SIN
    if _COS_SIN is None:
        half = DH // 2
        inv_freq = 1.0 / (10000.0 ** (np.arange(half, dtype=np.float32) / half))
        f = np.outer(np.arange(L, dtype=np.float32), inv_freq)
        cosF = np.concatenate([np.cos(f), np.cos(f)], -1).astype(BF).reshape(NT, 128, DH)
        sinF2 = np.concatenate([-np.sin(f), np.sin(f)], -1).astype(BF).reshape(NT, 128, DH)
        _COS_SIN = (cosF, sinF2)
    return _COS_SIN


F8 = ml_dtypes.float8_e4m3


def make_core_inputs(x, w_qkv, w_out, masks, b, g):
    xT = np.ascontiguousarray(x[b].T) * SX                  # [1024, 2048] scaled
    x8 = xT.astype(F8)
    rx8 = (xT - x8.astype(np.float32)).astype(F8)
    wg = np.concatenate([
        w_qkv[256 * g:256 * g + 256],
        w_qkv[1024 + 64 * g:1024 + 64 * g + 64],
        w_qkv[1280 + 64 * g:1280 + 64 * g + 64]], 0)        # [384, 1024]
    wT = np.ascontiguousarray(wg.T) * SW
    w8 = wT.astype(F8)
    rw8 = (wT - w8.astype(np.float32)).astype(F8)
    woutP = np.stack([
        np.ascontiguousarray(w_out[:, 256 * g + 128 * p:256 * g + 128 * p + 128].T)
        for p in range(2)]).astype(BF)                       # [2, 128, 1024]
    cosF, sinF2 = _cos_sin()
    return {
        "x8": x8.reshape(ND, 128, L), "rx8": rx8.reshape(ND, 128, L),
        "w8": w8.reshape(ND, 128, 384), "rw8": rw8.reshape(ND, 128, 384),
        "woutP": woutP,
        "cosF": cosF, "sinF2": sinF2,
        "masks": np.ascontiguousarray(masks[b]).astype(BF),
    }


_PROGRAM_CACHE = {}


def get_program(kts, widths):
    key = (tuple(tuple(k) for k in kts), tuple(tuple(w) for w in widths))
    if key not in _PROGRAM_CACHE:
        _PROGRAM_CACHE[key] = build_program(kts, widths)
    return _PROGRAM_CACHE[key]


def kernel(x, w_qkv, w_out, reset_mask):
    x = np.asarray(x, np.float32)
    w_qkv = np.asarray(w_qkv, np.float32)
    w_out = np.asarray(w_out, np.float32)
    reset_mask = np.asarray(reset_mask)

    kts, widths, masks = plan_structure(reset_mask)
    nc = get_program(kts, widths)
    if not getattr(nc, "_waitsplit_done", False):
        split_multi_waits(nc)
        nc._waitsplit_done = True

    in_maps = [make_core_inputs(x, w_qkv, w_out, masks, b, g)
               for b in range(B) for g in range(NG)]
    from concourse import bass_utils
    res = bass_utils.run_bass_kernel_spmd(nc, in_maps, core_ids=list(range(8)))

    out = x.copy()
    core = 0
    for b in range(B):
        acc = np.zeros((D, L), np.float32)
        for g in range(NG):
            acc += res.results[core]["outT"].astype(np.float32)
            core += 1
        out[b] += acc.T
    return out

